# revision 9
# baseline (speedup 1.0000x reference)
"""Causal Performer (linear attention) Trainium2 kernel.

Full inputs in, full output out. Sharding: data-parallel over batch (B=2)
x tensor-parallel over heads (16 heads -> 4 per core), 8 cores total.
Each core computes a partial output projection (its heads' rows of w_o);
the host sums the 4 partials per batch element.

Math notes vs the reference:
  - BLOCK_H=BLOCK_W=1 makes the blockwise selection an inclusive causal
    prefix scan -> standard chunked linear attention.
  - qp normalization cancels in numerator/denominator (up to an EPS-scale
    term ~1e-5 relative), so qp is used unnormalized.
  - kp normalization b[j] = 1/(sum_f kp + EPS) is folded into the value
    matrix v1[j] = b[j] * [v[j] | 1], which feeds both the intra-chunk
    (A_masked @ v1) and the running-state (kp^T @ v1) paths exactly once.
  - The feature-map projection is fused on the host:
    s = (q @ w_q^T)_head @ omega^T = q @ W2 with W2 = w_q_head^T @ omega^T.
"""

import numpy as np

L, D = 4096, 1024
H_ALL, DK, F = 16, 64, 16
HC = 4              # heads per core
M = HC * DK         # 256 head-dims per core
EPS = 1e-6
C = 128             # scan chunk
C2 = 512            # projection / load chunk
NC2 = L // C2       # 8
SUB = C2 // C       # 4
NCH = L // C        # 32 scan chunks
N_CORES = 8

_CACHE = {}


def _build_bass():
    from contextlib import ExitStack

    import concourse.bacc as bacc
    import concourse.tile as tile
    from concourse import mybir
    from concourse.masks import make_identity

    f16 = mybir.dt.float16
    f32 = mybir.dt.float32

    nc = bacc.Bacc("TRN2", debug=False, num_devices=1)
    q_d = nc.dram_tensor("qb", [L, D], f16, kind="ExternalInput").ap()
    k_d = nc.dram_tensor("kb", [L, D], f16, kind="ExternalInput").ap()
    v_d = nc.dram_tensor("vb", [L, D], f16, kind="ExternalInput").ap()
    # padded: head h's F features at columns 32h..32h+16 (32-aligned bases)
    w2q_d = nc.dram_tensor("w2q", [D, HC * 32], f16, kind="ExternalInput").ap()
    w2k_d = nc.dram_tensor("w2k", [D, HC * 32], f16, kind="ExternalInput").ap()
    wvT_d = nc.dram_tensor("wvT", [D, M], f16, kind="ExternalInput").ap()
    woT_d = nc.dram_tensor("woT", [M, D], f16, kind="ExternalInput").ap()
    out_d = nc.dram_tensor("out", [L, D], f32, kind="ExternalOutput").ap()

    KD = D // 128  # 8 contraction blocks

    with tile.TileContext(nc) as tc, ExitStack() as ctx:
        consts = ctx.enter_context(tc.tile_pool(name="consts", bufs=1))
        io = ctx.enter_context(tc.tile_pool(name="io", bufs=2))
        work = ctx.enter_context(tc.tile_pool(name="work", bufs=2))
        small = ctx.enter_context(tc.tile_pool(name="small", bufs=4))
        outp = ctx.enter_context(tc.tile_pool(name="outp", bufs=3))
        ps_s = ctx.enter_context(tc.tile_pool(name="ps_s", bufs=2, space="PSUM"))
        ps_w = ctx.enter_context(tc.tile_pool(name="ps_w", bufs=4, space="PSUM"))
        ps_o = ctx.enter_context(tc.tile_pool(name="ps_o", bufs=2, space="PSUM"))

        # --- constants ---
        w2q_sb = consts.tile([128, KD, HC * 32], f16)
        nc.sync.dma_start(out=w2q_sb, in_=w2q_d.rearrange("(o p) f -> p o f", p=128))
        w2k_sb = consts.tile([128, KD, HC * 32], f16)
        nc.sync.dma_start(out=w2k_sb, in_=w2k_d.rearrange("(o p) f -> p o f", p=128))
        wvT_sb = consts.tile([128, KD, M], f16)
        nc.sync.dma_start(out=wvT_sb, in_=wvT_d.rearrange("(o p) m -> p o m", p=128))
        woT_sb = consts.tile([128, M // 128, D], f16)
        nc.sync.dma_start(out=woT_sb, in_=woT_d.rearrange("(o p) n -> p o n", p=128))

        ident = consts.tile([128, 128], f16)
        make_identity(nc, ident)
        # causal mask over (j=key partition, i=query free): 1 where j <= i
        mask = consts.tile([128, 128], f32)
        nc.vector.memset(mask, 1.0)
        nc.gpsimd.affine_select(
            out=mask, in_=mask, pattern=[[1, 128]],
            compare_op=mybir.AluOpType.is_ge, fill=0.0,
            base=0, channel_multiplier=-1,
        )

        # persistent running state: [f, h, (v|1)] accumulated over chunks.
        # Kept in SBUF (not PSUM): multiple interleaved matmul accumulation
        # groups in one PSUM bank corrupt each other (start=True clears the
        # whole bank's has_written bits).
        state_acc = consts.tile([F, HC, DK + 1], f32)
        nc.vector.memset(state_acc, 0.0)

        for c2 in range(NC2):
            l0 = c2 * C2
            # --- transposed loads: [d_block, l] ---
            qT = io.tile([128, KD, C2], f16, tag="qT")
            kT = io.tile([128, KD, C2], f16, tag="kT")
            vT = io.tile([128, KD, C2], f16, tag="vT")
            for t_sb, t_d in ((qT, q_d), (kT, k_d), (vT, v_d)):
                for kd in range(KD):
                    nc.sync.dma_start_transpose(
                        out=t_sb[:, kd, :],
                        in_=t_d[l0:l0 + C2, kd * 128:(kd + 1) * 128],
                    )

            # --- fused feature projection: s = W2^T @ xT (heads at 32-aligned rows)
            sq_ps = ps_s.tile([128, C2], f32, tag="s")
            sk_ps = ps_s.tile([128, C2], f32, tag="s")
            for kd in range(KD):
                nc.tensor.matmul(
                    sq_ps, lhsT=w2q_sb[:, kd, :], rhs=qT[:, kd, :],
                    start=(kd == 0), stop=(kd == KD - 1),
                )
            for kd in range(KD):
                nc.tensor.matmul(
                    sk_ps, lhsT=w2k_sb[:, kd, :], rhs=kT[:, kd, :],
                    start=(kd == 0), stop=(kd == KD - 1),
                )

            sq_sq = work.tile([128, C2], f32, tag="sq_sq")
            nc.scalar.square(sq_sq, sq_ps)
            sk_sq = work.tile([128, C2], f32, tag="sk_sq")
            nc.scalar.square(sk_sq, sk_ps)

            # per-head feature maps exp(-0.5 s^2), kept transposed [f, l]
            qpT = []
            kpT = []
            for h in range(HC):
                qp_h = small.tile([F, C2], f16, tag=f"qpT{h}")
                nc.scalar.activation(
                    qp_h, sq_sq[h * 32:h * 32 + F, :],
                    mybir.ActivationFunctionType.Exp, scale=-0.5,
                )
                qpT.append(qp_h)
                kp_h = small.tile([F, C2], f16, tag=f"kpT{h}")
                nc.scalar.activation(
                    kp_h, sk_sq[h * 32:h * 32 + F, :],
                    mybir.ActivationFunctionType.Exp, scale=-0.5,
                )
                kpT.append(kp_h)

            for sub in range(SUB):
                c = c2 * SUB + sub
                ls = sub * C

                # kp natural layout [j, f] via PE transpose
                knat_ps = ps_w.tile([128, HC * F], f16, tag="w")
                for h in range(HC):
                    nc.tensor.transpose(
                        knat_ps[:, h * F:(h + 1) * F],
                        kpT[h][:, ls:ls + C],
                        ident[:F, :F],
                    )
                knat = small.tile([128, HC * F], f16, tag="knat")
                nc.scalar.copy(knat, knat_ps)

                # b[j] = 1/(sum_f kp + EPS) per head
                bsum = small.tile([128, HC], f32, tag="bsum")
                nc.vector.reduce_sum(
                    out=bsum,
                    in_=knat_ps.rearrange("p (h f) -> p h f", h=HC),
                    axis=mybir.AxisListType.X,
                )
                b4 = small.tile([128, HC], f32, tag="b4")
                nc.vector.tensor_scalar_add(b4, bsum, EPS)
                nc.vector.reciprocal(b4, b4)

                # vh natural [l, m]
                vh_ps = ps_w.tile([128, M], f32, tag="w")
                for kd in range(KD):
                    nc.tensor.matmul(
                        vh_ps, lhsT=vT[:, kd, ls:ls + C], rhs=wvT_sb[:, kd, :],
                        start=(kd == 0), stop=(kd == KD - 1),
                    )

                # v1 = b * [v | 1]
                v1 = small.tile([128, HC, DK + 1], f16, tag="v1")
                nc.vector.tensor_tensor(
                    v1[:, :, 0:DK],
                    vh_ps.rearrange("p (h d) -> p h d", h=HC),
                    b4[:, :, None].to_broadcast((128, HC, DK)),
                    mybir.AluOpType.mult,
                )
                nc.vector.tensor_copy(v1[:, :, DK], b4)

                oh = outp.tile([128, M], f16, tag="oh")
                for h in range(HC):
                    # A^T[j, i] = sum_f kp[j, f] qp[i, f]
                    a_ps = ps_w.tile([128, C], f32, tag="w")
                    nc.tensor.matmul(
                        a_ps, lhsT=kpT[h][:, ls:ls + C], rhs=qpT[h][:, ls:ls + C],
                        start=True, stop=True,
                    )
                    a_m = small.tile([128, C], f16, tag="a_m")
                    nc.vector.tensor_mul(a_m, a_ps, mask)

                    o_ps = ps_w.tile([128, DK + 1], f32, tag="w")
                    nc.tensor.matmul(
                        o_ps, lhsT=a_m, rhs=v1[:, h, :],
                        start=True, stop=(c == 0),
                    )
                    if c > 0:
                        st_sb = small.tile([F, DK + 1], f16, tag="st_sb")
                        nc.gpsimd.tensor_copy(st_sb, state_acc[:, h, :])
                        nc.tensor.matmul(
                            o_ps, lhsT=qpT[h][:, ls:ls + C], rhs=st_sb,
                            start=False, stop=True,
                        )
                    # state += kp^T @ v1  (delta in PSUM, accumulate in SBUF)
                    if c < NCH - 1:
                        d_ps = ps_w.tile([F, DK + 1], f32, tag="w")
                        nc.tensor.matmul(
                            d_ps, lhsT=knat[:, h * F:(h + 1) * F],
                            rhs=v1[:, h, :], start=True, stop=True,
                        )
                        nc.vector.tensor_add(
                            state_acc[:, h, :], state_acc[:, h, :], d_ps,
                        )

                    # out_h = num / (den + EPS)
                    r = small.tile([128, 1], f32, tag="r")
                    nc.vector.tensor_scalar_add(r, o_ps[:, DK:DK + 1], EPS)
                    nc.vector.reciprocal(r, r)
                    nc.vector.tensor_scalar_mul(
                        oh[:, h * DK:(h + 1) * DK], o_ps[:, 0:DK], r,
                    )

                # transpose oh -> [m, l] and project
                ohT = outp.tile([128, M // 128, C], f16, tag="ohT")
                for mb in range(M // 128):
                    ohT_ps = ps_w.tile([128, C], f16, tag="w")
                    nc.tensor.transpose(
                        ohT_ps, oh[:, mb * 128:(mb + 1) * 128], ident,
                    )
                    nc.scalar.copy(ohT[:, mb, :], ohT_ps)

                out_sb = outp.tile([128, D], f32, tag="out_sb")
                for nh in range(2):
                    op_ps = ps_o.tile([128, D // 2], f32, tag="op")
                    for mb in range(M // 128):
                        nc.tensor.matmul(
                            op_ps, lhsT=ohT[:, mb, :],
                            rhs=woT_sb[:, mb, nh * 512:(nh + 1) * 512],
                            start=(mb == 0), stop=(mb == M // 128 - 1),
                        )
                    if nh == 0:
                        nc.vector.tensor_copy(out_sb[:, 0:512], op_ps)
                    else:
                        nc.scalar.copy(out_sb[:, 512:1024], op_ps)
                nc.sync.dma_start(out=out_d[l0 + ls:l0 + ls + C, :], in_=out_sb)

    nc.compile()
    return nc


def _get_nc():
    if "nc" not in _CACHE:
        _CACHE["nc"] = _build_bass()
    return _CACHE["nc"]


def make_in_maps(q, k, v, w_q, w_k, w_v, w_o, omega):
    B = q.shape[0]
    in_maps = []
    for core in range(N_CORES):
        b = core // (N_CORES // B)
        g = core % (N_CORES // B)
        rows = slice(g * M, (g + 1) * M)
        om = omega.astype(np.float64)
        w2q = np.zeros((D, HC * 32), np.float64)
        w2k = np.zeros((D, HC * 32), np.float64)
        for h in range(HC):
            wq_h = w_q[rows][h * DK:(h + 1) * DK].astype(np.float64)  # [DK, D]
            wk_h = w_k[rows][h * DK:(h + 1) * DK].astype(np.float64)
            w2q[:, h * 32:h * 32 + F] = (om @ wq_h).T  # [D, F]
            w2k[:, h * 32:h * 32 + F] = (om @ wk_h).T
        in_maps.append({
            "qb": np.ascontiguousarray(q[b]).astype(np.float16),
            "kb": np.ascontiguousarray(k[b]).astype(np.float16),
            "vb": np.ascontiguousarray(v[b]).astype(np.float16),
            "w2q": np.ascontiguousarray(w2q).astype(np.float16),
            "w2k": np.ascontiguousarray(w2k).astype(np.float16),
            "wvT": np.ascontiguousarray(w_v[rows].T).astype(np.float16),
            "woT": np.ascontiguousarray(w_o[:, rows].T).astype(np.float16),
        })
    return in_maps


def kernel(q, k, v, w_q, w_k, w_v, w_o, omega):
    from concourse.bass_utils import run_bass_kernel_spmd

    B = q.shape[0]
    nc = _get_nc()
    in_maps = make_in_maps(q, k, v, w_q, w_k, w_v, w_o, omega)
    res = run_bass_kernel_spmd(nc, in_maps, core_ids=list(range(N_CORES)))
    out = np.zeros((B, L, D), np.float32)
    for core in range(N_CORES):
        out[core // (N_CORES // B)] += res.results[core]["out"]
    return out


# revision 16
# speedup vs baseline: 1.1603x; 1.1603x over previous
"""Causal Performer (linear attention) Trainium2 kernel.

Full inputs in, full output out. Sharding: data-parallel over batch (B=2)
x tensor-parallel over heads (16 heads -> 4 per core), 8 cores total.
Each core computes a partial output projection (its heads' rows of w_o);
the host sums the 4 partials per batch element.

Math notes vs the reference:
  - BLOCK_H=BLOCK_W=1 makes the blockwise selection an inclusive causal
    prefix scan -> standard chunked linear attention.
  - qp normalization cancels in numerator/denominator (up to an EPS-scale
    term ~1e-5 relative), so qp is used unnormalized.
  - kp normalization b[j] = 1/(sum_f kp + EPS) is folded into the value
    matrix v1[j] = b[j] * [v[j] | 1], which feeds both the intra-chunk
    (A_masked @ v1) and the running-state (kp^T @ v1) paths exactly once.
  - The feature-map projection is fused on the host:
    s = (q @ w_q^T)_head @ omega^T = q @ W2 with W2 = w_q_head^T @ omega^T.
"""

import numpy as np

L, D = 4096, 1024
H_ALL, DK, F = 16, 64, 16
HC = 4              # heads per core
M = HC * DK         # 256 head-dims per core
EPS = 1e-6
C = 128             # scan chunk
C2 = 512            # projection / load chunk
NC2 = L // C2       # 8
SUB = C2 // C       # 4
NCH = L // C        # 32 scan chunks
N_CORES = 8

_CACHE = {}


def _build_bass():
    from contextlib import ExitStack

    import concourse.bacc as bacc
    import concourse.tile as tile
    from concourse import mybir
    from concourse.masks import make_identity

    f16 = mybir.dt.float16
    f32 = mybir.dt.float32

    nc = bacc.Bacc("TRN2", debug=False, num_devices=1)
    q_d = nc.dram_tensor("qb", [L, D], f16, kind="ExternalInput").ap()
    k_d = nc.dram_tensor("kb", [L, D], f16, kind="ExternalInput").ap()
    v_d = nc.dram_tensor("vb", [L, D], f16, kind="ExternalInput").ap()
    # padded: head h's F features at columns 32h..32h+16 (32-aligned bases)
    w2q_d = nc.dram_tensor("w2q", [D, HC * 32], f16, kind="ExternalInput").ap()
    w2k_d = nc.dram_tensor("w2k", [D, HC * 32], f16, kind="ExternalInput").ap()
    wvT_d = nc.dram_tensor("wvT", [D, M], f16, kind="ExternalInput").ap()
    woT_d = nc.dram_tensor("woT", [M, D], f16, kind="ExternalInput").ap()
    out_d = nc.dram_tensor("out", [L, D], f32, kind="ExternalOutput").ap()

    KD = D // 128  # 8 contraction blocks

    with tile.TileContext(nc) as tc, ExitStack() as ctx:
        consts = ctx.enter_context(tc.tile_pool(name="consts", bufs=1))
        io = ctx.enter_context(tc.tile_pool(name="io", bufs=2))
        work = ctx.enter_context(tc.tile_pool(name="work", bufs=2))
        small = ctx.enter_context(tc.tile_pool(name="small", bufs=4))
        outp = ctx.enter_context(tc.tile_pool(name="outp", bufs=3))
        ps_s = ctx.enter_context(tc.tile_pool(name="ps_s", bufs=2, space="PSUM"))
        ps_w = ctx.enter_context(tc.tile_pool(name="ps_w", bufs=4, space="PSUM"))
        ps_o = ctx.enter_context(tc.tile_pool(name="ps_o", bufs=2, space="PSUM"))

        # --- constants ---
        w2q_sb = consts.tile([128, KD, HC * 32], f16)
        nc.sync.dma_start(out=w2q_sb, in_=w2q_d.rearrange("(o p) f -> p o f", p=128))
        w2k_sb = consts.tile([128, KD, HC * 32], f16)
        nc.sync.dma_start(out=w2k_sb, in_=w2k_d.rearrange("(o p) f -> p o f", p=128))
        wvT_sb = consts.tile([128, KD, M], f16)
        nc.sync.dma_start(out=wvT_sb, in_=wvT_d.rearrange("(o p) m -> p o m", p=128))
        woT_sb = consts.tile([128, M // 128, D], f16)
        nc.sync.dma_start(out=woT_sb, in_=woT_d.rearrange("(o p) n -> p o n", p=128))

        ident = consts.tile([128, 128], f16)
        make_identity(nc, ident)
        # causal mask over (j=key partition, i=query free): 1 where j <= i
        mask = consts.tile([128, 128], f32)
        nc.vector.memset(mask, 1.0)
        nc.gpsimd.affine_select(
            out=mask, in_=mask, pattern=[[1, 128]],
            compare_op=mybir.AluOpType.is_ge, fill=0.0,
            base=0, channel_multiplier=-1,
        )

        # persistent running state: [f, h, (v|1)] accumulated over chunks.
        # Kept in SBUF (not PSUM): multiple interleaved matmul accumulation
        # groups in one PSUM bank corrupt each other (start=True clears the
        # whole bank's has_written bits).
        state_acc = consts.tile([F, HC, DK + 1], f32)
        nc.vector.memset(state_acc, 0.0)

        LG = 2 * C2  # transpose-load granularity (bigger XBAR calls amortize
        qTg = kTg = vTg = None  # the ~1.2us fixed cost per DMA_TRANSPOSE)
        for c2 in range(NC2):
            l0 = c2 * C2
            # --- transposed loads: [d_block, l], issued every LG rows ---
            if c2 % (LG // C2) == 0:
                qTg = io.tile([128, KD, LG], f16, tag="qT")
                kTg = io.tile([128, KD, LG], f16, tag="kT")
                vTg = io.tile([128, KD, LG], f16, tag="vT")
                for t_sb, t_d in ((qTg, q_d), (kTg, k_d), (vTg, v_d)):
                    for kd in range(KD):
                        nc.sync.dma_start_transpose(
                            out=t_sb[:, kd, :],
                            in_=t_d[l0:l0 + LG, kd * 128:(kd + 1) * 128],
                        )
            off = (c2 % (LG // C2)) * C2
            qT = qTg[:, :, off:off + C2]
            kT = kTg[:, :, off:off + C2]
            vT = vTg[:, :, off:off + C2]

            # --- fused feature projection: s = W2^T @ xT (heads at 32-aligned rows)
            sq_ps = ps_s.tile([128, C2], f32, tag="s")
            sk_ps = ps_s.tile([128, C2], f32, tag="s")
            for kd in range(KD):
                nc.tensor.matmul(
                    sq_ps, lhsT=w2q_sb[:, kd, :], rhs=qT[:, kd, :],
                    start=(kd == 0), stop=(kd == KD - 1),
                )
            for kd in range(KD):
                nc.tensor.matmul(
                    sk_ps, lhsT=w2k_sb[:, kd, :], rhs=kT[:, kd, :],
                    start=(kd == 0), stop=(kd == KD - 1),
                )

            sq_sq = work.tile([128, C2], f32, tag="sq_sq")
            nc.scalar.square(sq_sq, sq_ps)
            sk_sq = work.tile([128, C2], f32, tag="sk_sq")
            nc.scalar.square(sk_sq, sk_ps)

            # per-head feature maps exp(-0.5 s^2), kept transposed [f, l]
            qpT = []
            kpT = []
            for h in range(HC):
                qp_h = small.tile([F, C2], f16, tag=f"qpT{h}")
                nc.scalar.activation(
                    qp_h, sq_sq[h * 32:h * 32 + F, :],
                    mybir.ActivationFunctionType.Exp, scale=-0.5,
                )
                qpT.append(qp_h)
                kp_h = small.tile([F, C2], f16, tag=f"kpT{h}")
                nc.scalar.activation(
                    kp_h, sk_sq[h * 32:h * 32 + F, :],
                    mybir.ActivationFunctionType.Exp, scale=-0.5,
                )
                kpT.append(kp_h)

            for sub in range(SUB):
                c = c2 * SUB + sub
                ls = sub * C

                # kp natural layout [j, f] via PE transpose; head h at cols
                # 32h..32h+16 (32-aligned so the batched state matmul's output
                # diagonal blocks land at legal partition bases)
                knat_ps = ps_w.tile([128, HC * 32], f16, tag="w")
                for h in range(HC):
                    nc.tensor.transpose(
                        knat_ps[:, h * 32:h * 32 + F],
                        kpT[h][:, ls:ls + C],
                        ident[:F, :F],
                    )
                knat = small.tile([128, HC * 32], f16, tag="knat")
                nc.scalar.copy(knat, knat_ps)

                # b[j] = 1/(sum_f kp + EPS) per head
                bsum = small.tile([128, HC], f32, tag="bsum")
                for h in range(HC):
                    nc.vector.reduce_sum(
                        out=bsum[:, h:h + 1],
                        in_=knat_ps[:, h * 32:h * 32 + F],
                        axis=mybir.AxisListType.X,
                    )
                b4 = small.tile([128, HC], f32, tag="b4")
                nc.vector.tensor_scalar_add(b4, bsum, EPS)
                nc.vector.reciprocal(b4, b4)

                # vh natural [l, m]
                vh_ps = ps_w.tile([128, M], f32, tag="w")
                for kd in range(KD):
                    nc.tensor.matmul(
                        vh_ps, lhsT=vT[:, kd, ls:ls + C], rhs=wvT_sb[:, kd, :],
                        start=(kd == 0), stop=(kd == KD - 1),
                    )

                # v1 = b * [v | 1]
                v1 = small.tile([128, HC, DK + 1], f16, tag="v1")
                nc.vector.tensor_tensor(
                    v1[:, :, 0:DK],
                    vh_ps.rearrange("p (h d) -> p h d", h=HC),
                    b4[:, :, None].to_broadcast((128, HC, DK)),
                    mybir.AluOpType.mult,
                )
                nc.vector.tensor_copy(v1[:, :, DK], b4)

                oh = outp.tile([128, M], f16, tag="oh")
                for h in range(HC):
                    # A^T[j, i] = sum_f kp[j, f] qp[i, f]
                    a_ps = ps_w.tile([128, C], f32, tag="w")
                    nc.tensor.matmul(
                        a_ps, lhsT=kpT[h][:, ls:ls + C], rhs=qpT[h][:, ls:ls + C],
                        start=True, stop=True,
                    )
                    a_m = small.tile([128, C], f16, tag="a_m")
                    nc.vector.tensor_mul(a_m, a_ps, mask)

                    o_ps = ps_w.tile([128, DK + 1], f32, tag="w")
                    nc.tensor.matmul(
                        o_ps, lhsT=a_m, rhs=v1[:, h, :],
                        start=True, stop=(c == 0),
                    )
                    if c > 0:
                        st_sb = small.tile([F, DK + 1], f16, tag="st_sb")
                        nc.gpsimd.tensor_copy(st_sb, state_acc[:, h, :])
                        nc.tensor.matmul(
                            o_ps, lhsT=qpT[h][:, ls:ls + C], rhs=st_sb,
                            start=False, stop=True,
                        )

                    # out_h = num / (den + EPS)
                    r = small.tile([128, 1], f32, tag="r")
                    nc.vector.tensor_scalar_add(r, o_ps[:, DK:DK + 1], EPS)
                    nc.vector.reciprocal(r, r)
                    nc.vector.tensor_scalar_mul(
                        oh[:, h * DK:(h + 1) * DK], o_ps[:, 0:DK], r,
                    )

                # state += kp^T @ v1: all heads in ONE matmul — off-diagonal
                # (h, h') blocks are computed but unread; diagonal blocks land
                # at partition base 32h (legal for the DVE adds)
                if c < NCH - 1:
                    d_ps = ps_w.tile([HC * 32, HC * (DK + 1)], f32, tag="w")
                    nc.tensor.matmul(
                        d_ps, lhsT=knat,
                        rhs=v1.rearrange("p h n -> p (h n)"),
                        start=True, stop=True,
                    )
                    for h in range(HC):
                        nc.vector.tensor_add(
                            state_acc[:, h, :], state_acc[:, h, :],
                            d_ps[h * 32:h * 32 + F, h * (DK + 1):(h + 1) * (DK + 1)],
                        )

                # transpose oh -> [m, l] and project
                ohT = outp.tile([128, M // 128, C], f16, tag="ohT")
                for mb in range(M // 128):
                    ohT_ps = ps_w.tile([128, C], f16, tag="w")
                    nc.tensor.transpose(
                        ohT_ps, oh[:, mb * 128:(mb + 1) * 128], ident,
                    )
                    nc.scalar.copy(ohT[:, mb, :], ohT_ps)

                out_sb = outp.tile([128, D], f32, tag="out_sb")
                for nh in range(2):
                    op_ps = ps_o.tile([128, D // 2], f32, tag="op")
                    for mb in range(M // 128):
                        nc.tensor.matmul(
                            op_ps, lhsT=ohT[:, mb, :],
                            rhs=woT_sb[:, mb, nh * 512:(nh + 1) * 512],
                            start=(mb == 0), stop=(mb == M // 128 - 1),
                        )
                    if nh == 0:
                        nc.vector.tensor_copy(out_sb[:, 0:512], op_ps)
                    else:
                        nc.scalar.copy(out_sb[:, 512:1024], op_ps)
                nc.scalar.dma_start(out=out_d[l0 + ls:l0 + ls + C, :], in_=out_sb)

    nc.compile()
    return nc


def _get_nc():
    if "nc" not in _CACHE:
        _CACHE["nc"] = _build_bass()
    return _CACHE["nc"]


def make_in_maps(q, k, v, w_q, w_k, w_v, w_o, omega):
    B = q.shape[0]
    in_maps = []
    for core in range(N_CORES):
        b = core // (N_CORES // B)
        g = core % (N_CORES // B)
        rows = slice(g * M, (g + 1) * M)
        om = omega.astype(np.float64)
        w2q = np.zeros((D, HC * 32), np.float64)
        w2k = np.zeros((D, HC * 32), np.float64)
        for h in range(HC):
            wq_h = w_q[rows][h * DK:(h + 1) * DK].astype(np.float64)  # [DK, D]
            wk_h = w_k[rows][h * DK:(h + 1) * DK].astype(np.float64)
            w2q[:, h * 32:h * 32 + F] = (om @ wq_h).T  # [D, F]
            w2k[:, h * 32:h * 32 + F] = (om @ wk_h).T
        in_maps.append({
            "qb": np.ascontiguousarray(q[b]).astype(np.float16),
            "kb": np.ascontiguousarray(k[b]).astype(np.float16),
            "vb": np.ascontiguousarray(v[b]).astype(np.float16),
            "w2q": np.ascontiguousarray(w2q).astype(np.float16),
            "w2k": np.ascontiguousarray(w2k).astype(np.float16),
            "wvT": np.ascontiguousarray(w_v[rows].T).astype(np.float16),
            "woT": np.ascontiguousarray(w_o[:, rows].T).astype(np.float16),
        })
    return in_maps


def kernel(q, k, v, w_q, w_k, w_v, w_o, omega):
    from concourse.bass_utils import run_bass_kernel_spmd

    B = q.shape[0]
    nc = _get_nc()
    in_maps = make_in_maps(q, k, v, w_q, w_k, w_v, w_o, omega)
    res = run_bass_kernel_spmd(nc, in_maps, core_ids=list(range(N_CORES)))
    out = np.zeros((B, L, D), np.float32)
    for core in range(N_CORES):
        out[core // (N_CORES // B)] += res.results[core]["out"]
    return out
